# revision 25
# baseline (speedup 1.0000x reference)
"""Multi-head self-attention (B=2, T=2048, D=1024, H=16) on 8 TRN2 NeuronCores.

Sharding: core c -> (b = c // 4, head-group hg = c % 4); each core computes the
full causal attention + partial output projection for its 4 heads of one batch
element.  The host pre-transposes x, pre-slices Wqkv columns / Wout rows per
head group, and sums the 4 bf16 partial projections per batch element (+ bout)
at the end.

v2 schedule (single TileContext, everything software-pipelined):
  - 8 warmup matmuls on a memset tile pre-warm the PE HAM clock gate during
    the initial DMA window.
  - q/k projection runs o-OUTER over T-quarters so each quarter only needs a
    prefix of the xT DMA stream (DMA stays ahead of the PE; no stalls).
  - S^T matmuls + exp run DECOUPLED ahead of the AV stream (run-ahead cursor):
    the exp work (the real bottleneck engine-wise) starts while the PE is
    still busy with projections.  Sub-diagonal blocks: ScalarE exact exp on
    head-half 0, VectorE one-op Schraudolph on head-half 1.  Diagonal blocks:
    exact ScalarE exp on live columns + one fused [2,128] triangle multiply.
  - AV accumulates per (query-tile, head-pair) with pair-OUTER psum rotation
    over two explicit rings, so a pair's softmax-finish (reciprocal + two
    half-crossing DMAs + psum*rec -> bf16 merged) overlaps the next pair's
    compute; no query-tile-boundary stalls.
  - The output projection (phase C) is interleaved per query-tile INTO the
    attention stream (emitted during the next tile's AV slots), with the
    out-DMA per (2 e-blocks, 512 tokens) chunk, so there is no serial tail.
  - All PSUM-drain copies are distributed over ScalarE/VectorE (GpSimd for
    SBUF-only ops) by a greedy load-balance model.
"""

import math
from contextlib import ExitStack

import numpy as np
import ml_dtypes

import concourse.bass as bass
import concourse.bacc as bacc_mod
import concourse.mybir as mybir
import concourse.tile as tile
FP32 = mybir.dt.float32
INT16 = mybir.dt.int16
BF16 = mybir.dt.bfloat16
AF = mybir.ActivationFunctionType
ALU = mybir.AluOpType

B, T, D, H = 2, 2048, 1024, 16
Dh = D // H          # 64
NCORES = 8
HPC = 4              # heads per core
NPAIR = HPC // 2     # head pairs per core (2 heads share a 128-partition block)
IT = T // 512        # 4 query tiles of 512
JB = T // 128        # 16 key blocks of 128
KO = D // 128        # 8 contraction blocks for the projections
SCALE = 1.0 / math.sqrt(Dh)

# Schraudolph bit-trick exp(s/8): i16 = round(s*A/2^16 + B/2^16), whose bit
# pattern IS the bf16 of exp(s/8) (~3% sawtooth).  Used only on sub-diagonal
# blocks where long-row averaging + consistent denominators make it
# indistinguishable from exact (verified end-to-end).
SCH_A16 = float((1 << 23) * (1.4426950408889634 / 8.0) / 65536.0)
SCH_B16 = float((127 * (1 << 23) - 365000) / 65536.0)


def build_program(compile=True):
    nc = bacc_mod.Bacc()

    xT = nc.declare_dram_parameter("xT", [D, T], BF16, isOutput=False)
    wqk = nc.declare_dram_parameter("wqk", [128, KO, 2 * HPC * Dh], BF16,
                                    isOutput=False)
    wv = nc.declare_dram_parameter("wv", [128, KO, HPC * Dh], BF16,
                                   isOutput=False)
    wout = nc.declare_dram_parameter("wout", [128, 2, D], BF16, isOutput=False)
    tri = nc.declare_dram_parameter("tri", [128, 256], BF16, isOutput=False)
    out = nc.declare_dram_parameter("outT", [D, T], BF16, isOutput=True)

    xT_r = xT.rearrange("(o p) t -> p o t", p=128)
    out_r = out.rearrange("(e p) t -> p e t", p=128)

    with ExitStack() as ctx:
        tc = ctx.enter_context(tile.TileContext(nc))
        persist = ctx.enter_context(tc.tile_pool(name="persist", bufs=1))
        pb = ctx.enter_context(tc.tile_pool(name="work", bufs=2))
        psb = ctx.enter_context(tc.tile_pool(name="ps", bufs=1, space="PSUM"))

        # ---------------- persistent tiles ----------------
        qkT = {}
        for nm in ("qT0", "qT1", "kT0", "kT1"):
            qkT[nm] = persist.tile([128, T], BF16, name=nm, tag=nm)
        V_aug = persist.tile([128, JB, HPC, 128], BF16, name="V_aug",
                             tag="V_aug")
        merged = [
            persist.tile([128, IT, 512], BF16, name=f"merged{p}",
                         tag=f"merged{p}")
            for p in range(NPAIR)
        ]
        wout_sb = persist.tile([128, 2, D], BF16, name="wout_sb", tag="wout_sb")
        tri_sb = persist.tile([128, 2, 128], BF16, name="tri_sb", tag="tri_sb")
        warm = persist.tile([128, 512], BF16, name="warm", tag="warm")
        # diagonal-class P^T tiles, double-buffered by it parity
        diag_pT = {
            (db, q, pr): persist.tile([128, 2, 512], BF16,
                                      name=f"pTd{db}_{q}_{pr}",
                                      tag=f"pTd{db}_{q}_{pr}")
            for db in range(2) for q in range(4) for pr in range(NPAIR)
        }

        xT_sb = pb.tile([128, KO, T], BF16, name="xT_sb", tag="xT_sb", bufs=1)
        wqk_sb = pb.tile([128, KO, 2 * HPC * Dh], BF16, name="wqk_sb",
                         tag="wqk_sb", bufs=1)
        wv_sb = pb.tile([128, KO, HPC * Dh], BF16, name="wv_sb", tag="wv_sb",
                        bufs=1)

        # ---------------- engine load balancer ----------------
        loads = {"s": 0.0, "v": 0.0, "g": 0.0}

        def cost(eng, elems):
            if eng == "s":
                return 110 + 1.0 * elems
            if eng == "v":
                return 140 + 1.05 * elems
            return 340 + 1.6 * elems

        def drain(dst, src, elems):
            """PSUM->SBUF copy on the lighter of ScalarE/VectorE."""
            if loads["s"] + cost("s", elems) <= loads["v"] + cost("v", elems):
                loads["s"] += cost("s", elems)
                nc.scalar.copy(dst, src)
            else:
                loads["v"] += cost("v", elems)
                nc.vector.tensor_copy(dst, src)

        # ---------------- memsets ----------------
        nc.gpsimd.memset(warm[:], 0.0)
        # split-ones: even heads [V|1], odd heads [1|V]
        nc.gpsimd.memset(V_aug[:, :, 0::2, 64:128], 1.0)
        nc.gpsimd.memset(V_aug[:, :, 1::2, 0:64], 1.0)
        for (db, q, pr), t_ in diag_pT.items():
            if q > 0:
                nc.gpsimd.memset(t_[:, :, : 128 * q], 0.0)

        # ---------------- DMAs (ordered to match consumption) ----------------
        # Q01 consumes (wqk[o], xT[o, 0:1024]) per o ascending; interleave so
        # no large transfer blocks the completion semaphore of an early need.
        for o in range(KO):
            nc.sync.dma_start(wqk_sb[:, o], wqk[:, o])
            nc.sync.dma_start(xT_sb[:, o, 0:1024], xT_r[:, o, 0:1024])
        nc.sync.dma_start(tri_sb[:], tri[:])
        nc.sync.dma_start(wv_sb[:], wv[:])
        for o in range(KO):
            nc.sync.dma_start(xT_sb[:, o, 1024:2048], xT_r[:, o, 1024:2048])
        nc.sync.dma_start(wout_sb[:], wout[:])

        # ---------------- PSUM rings ----------------
        # ring A/B: warmup, qk even quarters, S slots, C groups; during the
        # AV loop the CD tag not held by the live ctx accumulator joins the
        # ring (depth 3) to deepen the S->exp pipeline.
        # ring C/D: qk odd quarters, V tiles, AV ctx accumulators.
        nAB = [0]
        nCD = [0]
        free_cd = [None]

        def tileAB(name):
            tags = ["psA", "psB"] + ([free_cd[0]] if free_cd[0] else [])
            t = psb.tile([128, 2, 512], FP32, name=name,
                         tag=tags[nAB[0] % len(tags)], bufs=1)
            nAB[0] += 1
            return t

        def tileCD(name):
            t = psb.tile([128, 2, 512], FP32, name=name,
                         tag=f"ps{'CD'[nCD[0] % 2]}", bufs=1)
            nCD[0] += 1
            return t

        # ---------------- warmup (HAM pre-warm during DMA head) -------------
        wps = tileAB("warm_ps")
        for i in range(6):
            nc.tensor.matmul(wps[:, i % 2], lhsT=warm[:, 0:128], rhs=warm[:],
                             start=True, stop=True)

        # ---------------- q/k projection: o-outer over quarter-PAIRS --------
        # Two T-quarters share each LDWEIGHTS (the stationary wqk block),
        # halving the weight-load overhead per matmul; o-outer keeps the DMA
        # stream ahead of the PE.
        dests = [qkT["qT0"], qkT["qT1"], qkT["kT0"], qkT["kT1"]]

        def qk_pair(qp):
            qa, qb = 2 * qp, 2 * qp + 1
            sla = slice(512 * qa, 512 * (qa + 1))
            slb = slice(512 * qb, 512 * (qb + 1))
            t0a = tileAB(f"qk{qa}a")
            t0b = tileAB(f"qk{qa}b")
            t1a = tileCD(f"qk{qb}a")
            t1b = tileCD(f"qk{qb}b")
            for o in range(KO):
                for cb in range(4):
                    lhsT = wqk_sb[:, o, 128 * cb: 128 * (cb + 1)]
                    ta = t0a if cb < 2 else t0b
                    tb_ = t1a if cb < 2 else t1b
                    nc.tensor.matmul(ta[:, cb % 2], lhsT=lhsT,
                                     rhs=xT_sb[:, o, sla],
                                     start=(o == 0), stop=(o == KO - 1))
                    nc.tensor.matmul(tb_[:, cb % 2], lhsT=lhsT,
                                     rhs=xT_sb[:, o, slb],
                                     start=(o == 0), stop=(o == KO - 1))
            for cb in range(4):
                ta = t0a if cb < 2 else t0b
                tb_ = t1a if cb < 2 else t1b
                drain(dests[cb][:, sla], ta[:, cb % 2], 512)
                drain(dests[cb][:, slb], tb_[:, cb % 2], 512)

        qk_pair(0)  # quarters 0,1 -> qT/kT columns 0:1024

        # ---------------- S/exp run-ahead emitter ----------------
        slots = [(it, pair, jb)
                 for it in range(IT)
                 for pair in range(NPAIR)
                 for jb in range(4 * it + 4)]
        pTs = {}
        s_cur = [0]
        av_n = [0]

        def dve_exp(pT_ap, ps_ap):
            nc.vector.tensor_scalar(
                out=pT_ap.bitcast(INT16), in0=ps_ap,
                scalar1=SCH_A16, scalar2=SCH_B16,
                op0=ALU.mult, op1=ALU.add,
            )

        def emit_S(n):
            it, pair, jb = slots[n]
            qv = jb - 4 * it
            jsl = slice(128 * jb, 128 * (jb + 1))
            ps2 = tileAB(f"s{n}")
            if qv < 0:
                c0 = 0
                pT = pb.tile([128, 2, 512], BF16, name="pT", tag="pT_full",
                             bufs=24)
            else:
                c0 = 128 * qv
                pT = diag_pT[(it % 2, qv, pair)]
            isl = slice(512 * it + c0, 512 * (it + 1))
            for hl in range(2):
                rows = slice(64 * hl, 64 * (hl + 1))
                nc.tensor.matmul(
                    ps2[:, hl, c0:],
                    lhsT=qkT[f"kT{pair}"][rows, jsl],
                    rhs=qkT[f"qT{pair}"][rows, isl],
                    start=True, stop=True,
                )
            if qv < 0:
                # full-slot single-op exp on the lighter engine: exact on
                # ScalarE, one-op Schraudolph on VectorE (sub-diagonal only;
                # long-row averaging keeps the sawtooth invisible end-to-end)
                ca = loads["s"] + cost("s", 1024)
                cv = loads["v"] + cost("v", 1024)
                if ca <= cv:
                    loads["s"] = ca
                    nc.scalar.activation(pT[:, :, :], ps2[:, :, :], AF.Exp,
                                         scale=SCALE)
                else:
                    loads["v"] = cv
                    dve_exp(pT[:, :, :], ps2[:, :, :])
            else:
                nc.scalar.activation(pT[:, :, c0:], ps2[:, :, c0:], AF.Exp,
                                     scale=SCALE)
                loads["s"] += cost("s", 2 * (512 - c0))
                # fused triangle multiply over both head-halves
                el = 256
                if loads["g"] + cost("g", el) <= loads["v"] + cost("v", el):
                    loads["g"] += cost("g", el)
                    eng = nc.gpsimd
                else:
                    loads["v"] += cost("v", el)
                    eng = nc.vector
                eng.tensor_tensor(
                    out=pT[:, :, c0:c0 + 128], in0=pT[:, :, c0:c0 + 128],
                    in1=tri_sb[:], op=ALU.mult,
                )
            pTs[n] = pT

        # run-ahead caps: a pre-emitted S slot must never depend (via psum /
        # pT-pool / diag-buffer WAR) on an AV emitted later, or the in-order
        # engine streams deadlock.
        full_alloc = [0]
        full_freed = [0]     # bumped when an AV consuming a pT_full is emitted
        END_OF_IT = [8, 24, 48, 80]

        def allowed(k):
            it, pair, jb = slots[k]
            if jb - 4 * it < 0:           # sub-diagonal: pT_full pool bound
                return full_alloc[0] - full_freed[0] < 22
            # diagonal: double-buffered by it parity; it's buffer was last
            # used by it-2, whose AVs must already be emitted
            return it < 2 or av_n[0] >= END_OF_IT[it - 2]

        def top_up(n):
            while s_cur[0] < min(n, len(slots)) and allowed(s_cur[0]):
                it, pair, jb = slots[s_cur[0]]
                if jb - 4 * it < 0:
                    full_alloc[0] += 1
                emit_S(s_cur[0])
                s_cur[0] += 1

        # ---------------- V projection (packed psum tiles) -------------------
        # V half 1 (tb 0..7, needs xT cols 0:1024 only) runs right after the
        # first quarter-pair, with the it0/it1 S+exp pre-run interleaved; then
        # the second quarter-pair; then V half 2 with the it2 pre-run.
        def v_half(vh, pre):
            for vt in range(2 * vh, 2 * vh + 2):
                vtile = tileCD(f"v{vt}")
                for tb in range(4 * vt, 4 * vt + 4):
                    psv = vtile[:, (tb % 4) // 2,
                                256 * (tb % 2): 256 * (tb % 2) + 256]
                    for o in range(KO):
                        nc.tensor.matmul(
                            psv,
                            lhsT=xT_sb[:, o, 128 * tb: 128 * (tb + 1)],
                            rhs=wv_sb[:, o],
                            start=(o == 0), stop=(o == KO - 1),
                        )
                    psv_r = psv.rearrange("p (h d) -> p h d", h=HPC)
                    drain(V_aug[:, tb, 0::2, 0:64], psv_r[:, 0::2, :], 128)
                    drain(V_aug[:, tb, 1::2, 64:128], psv_r[:, 1::2, :], 128)
                    top_up(pre[tb])

        top_up(8)                    # it0 S+exp (diag tiles only)
        v_half(0, {0: 10, 1: 12, 2: 14, 3: 16, 4: 18, 5: 20, 6: 22, 7: 24})
        qk_pair(1)                   # quarters 2,3
        v_half(1, {8: 27, 9: 30, 10: 33, 11: 36, 12: 39, 13: 42, 14: 45,
                   15: 48})

        # ---------------- AV + finish + interleaved C ----------------
        def emit_AV(it, pair, jb, pT, ctx_t):
            njb = 4 * it + 4
            c0 = max(0, 128 * (jb - 4 * it))
            for hl in range(2):
                h = 2 * pair + hl
                nc.tensor.matmul(
                    ctx_t[:, hl, c0:],
                    lhsT=V_aug[:, jb, h, :],
                    rhs=pT[:, hl, c0:],
                    start=(jb == 0), stop=(jb == njb - 1),
                )

        def finish(it, pair, ctx_t):
            # release the ctx psum banks fast: reciprocal (VectorE) runs
            # concurrently with both unnormalized-ctx copies (ScalarE); the
            # normalize then happens entirely off-PSUM on GpSimd, in place.
            recs = pb.tile([128, 2, 512], FP32, name="recs", tag="recs",
                           bufs=2)
            nc.vector.reciprocal_approx_fast(recs[:], ctx_t[:])
            loads["v"] += cost("v", 1024)
            nc.scalar.copy(merged[pair][0:64, it], ctx_t[0:64, 0, :])
            nc.scalar.copy(merged[pair][64:128, it], ctx_t[64:128, 1, :])
            loads["s"] += 2 * cost("s", 512)
            rcn = pb.tile([128, 512], FP32, name="rec_n", tag="rec_n",
                          bufs=2)
            # half-crossing DMAs from the GpSimd queue (whose multiply
            # consumes them next) so they never head-of-line block the Sync
            # queue's output DMAs.
            nc.gpsimd.dma_start(rcn[0:64, :], recs[64:128, 0, :])
            nc.gpsimd.dma_start(rcn[64:128, :], recs[0:64, 1, :])
            nc.gpsimd.tensor_tensor(out=merged[pair][:, it],
                                    in0=merged[pair][:, it], in1=rcn[:],
                                    op=ALU.mult)
            loads["g"] += cost("g", 512)

        def emit_C_group(it_, gi):
            tpc = tileAB(f"c{it_}_{gi}")
            for pair in range(NPAIR):
                for e2 in range(2):
                    eb = 2 * gi + e2
                    nc.tensor.matmul(
                        tpc[:, e2],
                        lhsT=wout_sb[:, pair, 128 * eb: 128 * (eb + 1)],
                        rhs=merged[pair][:, it_],
                        start=(pair == 0), stop=(pair == NPAIR - 1),
                    )
            osb = pb.tile([128, 2, 512], BF16, name="osb", tag="osb", bufs=6)
            drain(osb[:], tpc[:], 1024)
            nc.sync.dma_start(
                out_r[:, 2 * gi: 2 * gi + 2, 512 * it_: 512 * (it_ + 1)],
                osb[:],
            )

        L = 20
        cq = []
        for it in range(IT):
            njb = 4 * it + 4
            for pair in range(NPAIR):
                ctx_tag = "CD"[nCD[0] % 2]
                ctx_t = tileCD(f"ctx{it}_{pair}")
                # the vacated CD tag is still draining through the previous
                # pair's finish chain; let it re-enter the S ring only a few
                # slots into this run.
                free_cd[0] = None
                for jb in range(njb):
                    n = av_n[0]
                    if jb < 4 * it:
                        full_freed[0] += 1
                    emit_AV(it, pair, jb, pTs.pop(n), ctx_t)
                    av_n[0] += 1
                    if jb == 2:
                        free_cd[0] = f"ps{'DC'['CD'.index(ctx_tag)]}"
                    top_up(n + 1 + L)
                    # hold back two C groups before the last it so the PE has
                    # ready work to chew on during the final finish chain
                    if cq and jb % 4 == 3 and (it < 3 or len(cq) > 1):
                        emit_C_group(*cq.pop(0))
                finish(it, pair, ctx_t)
                if pair == NPAIR - 1:
                    for gi in range(4):
                        cq.append((it, gi))
        while cq:
            emit_C_group(*cq.pop(0))

    if compile:
        nc.compile()
    return nc


_PROGRAM = None


def _get_program():
    global _PROGRAM
    if _PROGRAM is None:
        _PROGRAM = build_program()
    return _PROGRAM


def _tri():
    dj = np.arange(128)[:, None]
    di = np.arange(128)[None, :]
    t = (dj <= di).astype(ml_dtypes.bfloat16)
    return np.ascontiguousarray(np.concatenate([t, t], axis=1))


def make_in_maps(x, Wqkv, Wout):
    in_maps = []
    for core in range(NCORES):
        b, hg = core // (NCORES // B), core % (NCORES // B)
        c0 = hg * HPC * Dh
        csl = slice(c0, c0 + HPC * Dh)
        wqk_full = np.concatenate(
            [Wqkv[:, csl], Wqkv[:, D + c0: D + c0 + HPC * Dh]], axis=1
        ).astype(ml_dtypes.bfloat16)
        wv_full = Wqkv[:, 2 * D + c0: 2 * D + c0 + HPC * Dh].astype(
            ml_dtypes.bfloat16)
        in_maps.append({
            "tri": _tri(),
            "xT": np.ascontiguousarray(x[b].T).astype(ml_dtypes.bfloat16),
            "wqk": np.ascontiguousarray(
                wqk_full.reshape(KO, 128, 2 * HPC * Dh).transpose(1, 0, 2)),
            "wv": np.ascontiguousarray(
                wv_full.reshape(KO, 128, HPC * Dh).transpose(1, 0, 2)),
            "wout": np.ascontiguousarray(
                Wout[csl, :].astype(ml_dtypes.bfloat16)
                .reshape(2, 128, D).transpose(1, 0, 2)),
        })
    return in_maps


def kernel(x, causal_mask, key_padding_mask, Wqkv, bqkv, Wout, bout,
           _trace=False):
    from concourse.bass_utils import run_bass_kernel_spmd

    x = np.asarray(x, dtype=np.float32)
    Wqkv = np.asarray(Wqkv, dtype=np.float32)
    Wout = np.asarray(Wout, dtype=np.float32)
    bqkv = np.asarray(bqkv, dtype=np.float32)
    bout = np.asarray(bout, dtype=np.float32)
    if np.any(np.asarray(key_padding_mask)):
        raise NotImplementedError("key_padding_mask with padded keys")
    if np.any(bqkv):
        raise NotImplementedError("nonzero bqkv")

    nc = _get_program()
    in_maps = make_in_maps(x, Wqkv, Wout)
    res = run_bass_kernel_spmd(nc, in_maps, core_ids=list(range(NCORES)),
                               trace=_trace)
    G = NCORES // B
    outp = np.empty((B, T, D), dtype=np.float32)
    for b in range(B):
        acc = res.results[b * G]["outT"].astype(np.float32)
        for hg in range(1, G):
            acc += res.results[b * G + hg]["outT"].astype(np.float32)
        outp[b] = acc.T + bout
    kernel.last_exec_time_ns = res.exec_time_ns
    return outp


# revision 26
# speedup vs baseline: 1.1825x; 1.1825x over previous
"""Multi-head self-attention (B=2, T=2048, D=1024, H=16) on 8 TRN2 NeuronCores.

Sharding: core c -> (b = c // 4, head-group hg = c % 4); each core computes the
full causal attention + partial output projection for its 4 heads of one batch
element.  The host pre-transposes x, pre-slices Wqkv columns / Wout rows per
head group, and sums the 4 bf16 partial projections per batch element (+ bout)
at the end.

v2 schedule (single TileContext, everything software-pipelined):
  - 8 warmup matmuls on a memset tile pre-warm the PE HAM clock gate during
    the initial DMA window.
  - q/k projection runs o-OUTER over T-quarters so each quarter only needs a
    prefix of the xT DMA stream (DMA stays ahead of the PE; no stalls).
  - S^T matmuls + exp run DECOUPLED ahead of the AV stream (run-ahead cursor):
    the exp work (the real bottleneck engine-wise) starts while the PE is
    still busy with projections.  Sub-diagonal blocks: ScalarE exact exp on
    head-half 0, VectorE one-op Schraudolph on head-half 1.  Diagonal blocks:
    exact ScalarE exp on live columns + one fused [2,128] triangle multiply.
  - AV accumulates per (query-tile, head-pair) with pair-OUTER psum rotation
    over two explicit rings, so a pair's softmax-finish (reciprocal + two
    half-crossing DMAs + psum*rec -> bf16 merged) overlaps the next pair's
    compute; no query-tile-boundary stalls.
  - The output projection (phase C) is interleaved per query-tile INTO the
    attention stream (emitted during the next tile's AV slots), with the
    out-DMA per (2 e-blocks, 512 tokens) chunk, so there is no serial tail.
  - All PSUM-drain copies are distributed over ScalarE/VectorE (GpSimd for
    SBUF-only ops) by a greedy load-balance model.
"""

import math
from contextlib import ExitStack

import numpy as np
import ml_dtypes

import concourse.bass as bass
import concourse.bacc as bacc_mod
import concourse.mybir as mybir
import concourse.tile as tile
FP32 = mybir.dt.float32
INT16 = mybir.dt.int16
BF16 = mybir.dt.bfloat16
AF = mybir.ActivationFunctionType
ALU = mybir.AluOpType

B, T, D, H = 2, 2048, 1024, 16
Dh = D // H          # 64
NCORES = 8
HPC = 4              # heads per core
NPAIR = HPC // 2     # head pairs per core (2 heads share a 128-partition block)
IT = T // 512        # 4 query tiles of 512
JB = T // 128        # 16 key blocks of 128
KO = D // 128        # 8 contraction blocks for the projections
SCALE = 1.0 / math.sqrt(Dh)

# Schraudolph bit-trick exp(s/8): i16 = round(s*A/2^16 + B/2^16), whose bit
# pattern IS the bf16 of exp(s/8) (~3% sawtooth).  Used only on sub-diagonal
# blocks where long-row averaging + consistent denominators make it
# indistinguishable from exact (verified end-to-end).
SCH_A16 = float((1 << 23) * (1.4426950408889634 / 8.0) / 65536.0)
SCH_B16 = float((127 * (1 << 23) - 365000) / 65536.0)


def build_program(compile=True):
    nc = bacc_mod.Bacc()

    xT = nc.declare_dram_parameter("xT", [D, T], BF16, isOutput=False)
    wqk = nc.declare_dram_parameter("wqk", [128, KO, 2 * HPC * Dh], BF16,
                                    isOutput=False)
    wv = nc.declare_dram_parameter("wv", [128, KO, HPC * Dh], BF16,
                                   isOutput=False)
    wout = nc.declare_dram_parameter("wout", [128, 2, D], BF16, isOutput=False)
    tri = nc.declare_dram_parameter("tri", [128, 256], BF16, isOutput=False)
    out = nc.declare_dram_parameter("outT", [D, T], BF16, isOutput=True)

    xT_r = xT.rearrange("(o p) t -> p o t", p=128)
    out_r = out.rearrange("(e p) t -> p e t", p=128)

    with ExitStack() as ctx:
        tc = ctx.enter_context(tile.TileContext(nc))
        persist = ctx.enter_context(tc.tile_pool(name="persist", bufs=1))
        pb = ctx.enter_context(tc.tile_pool(name="work", bufs=2))
        psb = ctx.enter_context(tc.tile_pool(name="ps", bufs=1, space="PSUM"))

        # ---------------- persistent tiles ----------------
        qkT = {}
        for nm in ("qT0", "qT1", "kT0", "kT1"):
            qkT[nm] = persist.tile([128, T], BF16, name=nm, tag=nm)
        V_aug = persist.tile([128, JB, HPC, 128], BF16, name="V_aug",
                             tag="V_aug")
        merged = [
            persist.tile([128, IT, 512], BF16, name=f"merged{p}",
                         tag=f"merged{p}")
            for p in range(NPAIR)
        ]
        wout_sb = persist.tile([128, 2, D], BF16, name="wout_sb", tag="wout_sb")
        tri_sb = persist.tile([128, 2, 128], BF16, name="tri_sb", tag="tri_sb")
        warm = persist.tile([128, 512], BF16, name="warm", tag="warm")
        # diagonal-class P^T tiles, double-buffered by it parity
        diag_pT = {
            (db, q, pr): persist.tile([128, 2, 512], BF16,
                                      name=f"pTd{db}_{q}_{pr}",
                                      tag=f"pTd{db}_{q}_{pr}")
            for db in range(2) for q in range(4) for pr in range(NPAIR)
        }

        xT_sb = pb.tile([128, KO, T], BF16, name="xT_sb", tag="xT_sb", bufs=1)
        wqk_sb = pb.tile([128, KO, 2 * HPC * Dh], BF16, name="wqk_sb",
                         tag="wqk_sb", bufs=1)
        wv_sb = pb.tile([128, KO, HPC * Dh], BF16, name="wv_sb", tag="wv_sb",
                        bufs=1)

        # ---------------- engine load balancer ----------------
        loads = {"s": 0.0, "v": 0.0, "g": 0.0}

        def cost(eng, elems):
            if eng == "s":
                return 110 + 1.0 * elems
            if eng == "v":
                return 140 + 1.05 * elems
            return 340 + 1.6 * elems

        def drain(dst, src, elems):
            """PSUM->SBUF copy on the lighter of ScalarE/VectorE."""
            if loads["s"] + cost("s", elems) <= loads["v"] + cost("v", elems):
                loads["s"] += cost("s", elems)
                nc.scalar.copy(dst, src)
            else:
                loads["v"] += cost("v", elems)
                nc.vector.tensor_copy(dst, src)

        # ---------------- memsets ----------------
        nc.gpsimd.memset(warm[:], 0.0)
        # split-ones: even heads [V|1], odd heads [1|V]
        nc.gpsimd.memset(V_aug[:, :, 0::2, 64:128], 1.0)
        nc.gpsimd.memset(V_aug[:, :, 1::2, 0:64], 1.0)
        for (db, q, pr), t_ in diag_pT.items():
            if q > 0:
                nc.gpsimd.memset(t_[:, :, : 128 * q], 0.0)

        # ---------------- DMAs (ordered to match consumption) ----------------
        # Q01 consumes (wqk[o], xT[o, 0:1024]) per o ascending; interleave so
        # no large transfer blocks the completion semaphore of an early need.
        for o in range(KO):
            nc.sync.dma_start(wqk_sb[:, o], wqk[:, o])
            nc.sync.dma_start(xT_sb[:, o, 0:1024], xT_r[:, o, 0:1024])
        nc.sync.dma_start(tri_sb[:], tri[:])
        nc.sync.dma_start(wv_sb[:], wv[:])
        for o in range(KO):
            nc.sync.dma_start(xT_sb[:, o, 1024:2048], xT_r[:, o, 1024:2048])
        nc.sync.dma_start(wout_sb[:], wout[:])

        # ---------------- PSUM rings ----------------
        # ring A/B: warmup, qk even quarters, S slots, C groups; during the
        # AV loop the CD tag not held by the live ctx accumulator joins the
        # ring (depth 3) to deepen the S->exp pipeline.
        # ring C/D: qk odd quarters, V tiles, AV ctx accumulators.
        nAB = [0]
        nCD = [0]
        free_cd = [None]

        def tileAB(name):
            tags = ["psA", "psB"] + ([free_cd[0]] if free_cd[0] else [])
            t = psb.tile([128, 2, 512], FP32, name=name,
                         tag=tags[nAB[0] % len(tags)], bufs=1)
            nAB[0] += 1
            return t

        def tileCD(name):
            t = psb.tile([128, 2, 512], FP32, name=name,
                         tag=f"ps{'CD'[nCD[0] % 2]}", bufs=1)
            nCD[0] += 1
            return t

        # ---------------- warmup (HAM pre-warm during DMA head) -------------
        wps = tileAB("warm_ps")
        for i in range(6):
            nc.tensor.matmul(wps[:, i % 2], lhsT=warm[:, 0:128], rhs=warm[:],
                             start=True, stop=True)

        # ---------------- q/k projection: o-outer over quarter-PAIRS --------
        # Two T-quarters share each LDWEIGHTS (the stationary wqk block),
        # halving the weight-load overhead per matmul; o-outer keeps the DMA
        # stream ahead of the PE.
        dests = [qkT["qT0"], qkT["qT1"], qkT["kT0"], qkT["kT1"]]

        def qk_pair(qp):
            qa, qb = 2 * qp, 2 * qp + 1
            sla = slice(512 * qa, 512 * (qa + 1))
            slb = slice(512 * qb, 512 * (qb + 1))
            t0a = tileAB(f"qk{qa}a")
            t0b = tileAB(f"qk{qa}b")
            t1a = tileCD(f"qk{qb}a")
            t1b = tileCD(f"qk{qb}b")
            for o in range(KO):
                for cb in range(4):
                    lhsT = wqk_sb[:, o, 128 * cb: 128 * (cb + 1)]
                    ta = t0a if cb < 2 else t0b
                    tb_ = t1a if cb < 2 else t1b
                    nc.tensor.matmul(ta[:, cb % 2], lhsT=lhsT,
                                     rhs=xT_sb[:, o, sla],
                                     start=(o == 0), stop=(o == KO - 1))
                    nc.tensor.matmul(tb_[:, cb % 2], lhsT=lhsT,
                                     rhs=xT_sb[:, o, slb],
                                     start=(o == 0), stop=(o == KO - 1))
            for cb in range(4):
                ta = t0a if cb < 2 else t0b
                tb_ = t1a if cb < 2 else t1b
                drain(dests[cb][:, sla], ta[:, cb % 2], 512)
                drain(dests[cb][:, slb], tb_[:, cb % 2], 512)

        qk_pair(0)  # quarters 0,1 -> qT/kT columns 0:1024

        # ---------------- S/exp run-ahead emitter ----------------
        slots = [(it, pair, jb)
                 for it in range(IT)
                 for pair in range(NPAIR)
                 for jb in range(4 * it + 4)]
        pTs = {}
        s_cur = [0]
        av_n = [0]

        def dve_exp(pT_ap, ps_ap):
            nc.vector.tensor_scalar(
                out=pT_ap.bitcast(INT16), in0=ps_ap,
                scalar1=SCH_A16, scalar2=SCH_B16,
                op0=ALU.mult, op1=ALU.add,
            )

        def emit_S(n):
            it, pair, jb = slots[n]
            qv = jb - 4 * it
            jsl = slice(128 * jb, 128 * (jb + 1))
            ps2 = tileAB(f"s{n}")
            if qv < 0:
                c0 = 0
                pT = pb.tile([128, 2, 512], BF16, name="pT", tag="pT_full",
                             bufs=18)
            else:
                c0 = 128 * qv
                pT = diag_pT[(it % 2, qv, pair)]
            isl = slice(512 * it + c0, 512 * (it + 1))
            for hl in range(2):
                rows = slice(64 * hl, 64 * (hl + 1))
                nc.tensor.matmul(
                    ps2[:, hl, c0:],
                    lhsT=qkT[f"kT{pair}"][rows, jsl],
                    rhs=qkT[f"qT{pair}"][rows, isl],
                    start=True, stop=True,
                )
            if qv < 0:
                # full-slot single-op exp on the lighter engine: exact on
                # ScalarE, one-op Schraudolph on VectorE (sub-diagonal only;
                # long-row averaging keeps the sawtooth invisible end-to-end)
                ca = loads["s"] + cost("s", 1024)
                cv = loads["v"] + cost("v", 1024)
                if ca <= cv:
                    loads["s"] = ca
                    nc.scalar.activation(pT[:, :, :], ps2[:, :, :], AF.Exp,
                                         scale=SCALE)
                else:
                    loads["v"] = cv
                    dve_exp(pT[:, :, :], ps2[:, :, :])
            else:
                nc.scalar.activation(pT[:, :, c0:], ps2[:, :, c0:], AF.Exp,
                                     scale=SCALE)
                loads["s"] += cost("s", 2 * (512 - c0))
                # fused triangle multiply over both head-halves
                el = 256
                if loads["g"] + cost("g", el) <= loads["v"] + cost("v", el):
                    loads["g"] += cost("g", el)
                    eng = nc.gpsimd
                else:
                    loads["v"] += cost("v", el)
                    eng = nc.vector
                eng.tensor_tensor(
                    out=pT[:, :, c0:c0 + 128], in0=pT[:, :, c0:c0 + 128],
                    in1=tri_sb[:], op=ALU.mult,
                )
            pTs[n] = pT

        # run-ahead caps: a pre-emitted S slot must never depend (via psum /
        # pT-pool / diag-buffer WAR) on an AV emitted later, or the in-order
        # engine streams deadlock.
        full_alloc = [0]
        full_freed = [0]     # bumped when an AV consuming a pT_full is emitted
        END_OF_IT = [8, 24, 48, 80]

        def allowed(k):
            it, pair, jb = slots[k]
            if jb - 4 * it < 0:           # sub-diagonal: pT_full pool bound
                return full_alloc[0] - full_freed[0] < 16
            # diagonal: double-buffered by it parity; it's buffer was last
            # used by it-2, whose AVs must already be emitted
            return it < 2 or av_n[0] >= END_OF_IT[it - 2]

        def top_up(n):
            while s_cur[0] < min(n, len(slots)) and allowed(s_cur[0]):
                it, pair, jb = slots[s_cur[0]]
                if jb - 4 * it < 0:
                    full_alloc[0] += 1
                emit_S(s_cur[0])
                s_cur[0] += 1

        # ---------------- V projection (packed psum tiles) -------------------
        # V half 1 (tb 0..7, needs xT cols 0:1024 only) runs right after the
        # first quarter-pair, with the it0/it1 S+exp pre-run interleaved; then
        # the second quarter-pair; then V half 2 with the it2 pre-run.
        def v_half(vh, pre):
            for vt in range(2 * vh, 2 * vh + 2):
                vtile = tileCD(f"v{vt}")
                for tb in range(4 * vt, 4 * vt + 4):
                    psv = vtile[:, (tb % 4) // 2,
                                256 * (tb % 2): 256 * (tb % 2) + 256]
                    for o in range(KO):
                        nc.tensor.matmul(
                            psv,
                            lhsT=xT_sb[:, o, 128 * tb: 128 * (tb + 1)],
                            rhs=wv_sb[:, o],
                            start=(o == 0), stop=(o == KO - 1),
                        )
                    psv_r = psv.rearrange("p (h d) -> p h d", h=HPC)
                    drain(V_aug[:, tb, 0::2, 0:64], psv_r[:, 0::2, :], 128)
                    drain(V_aug[:, tb, 1::2, 64:128], psv_r[:, 1::2, :], 128)
                    top_up(pre[tb])

        top_up(8)                    # it0 S+exp (diag tiles only)
        v_half(0, {0: 10, 1: 12, 2: 14, 3: 16, 4: 18, 5: 20, 6: 22, 7: 24})
        qk_pair(1)                   # quarters 2,3
        v_half(1, {8: 27, 9: 30, 10: 33, 11: 36, 12: 39, 13: 42, 14: 45,
                   15: 48})

        # ---------------- AV + finish + interleaved C ----------------
        def emit_AV(it, pair, jb, pT, ctx_t):
            njb = 4 * it + 4
            c0 = max(0, 128 * (jb - 4 * it))
            for hl in range(2):
                h = 2 * pair + hl
                nc.tensor.matmul(
                    ctx_t[:, hl, c0:],
                    lhsT=V_aug[:, jb, h, :],
                    rhs=pT[:, hl, c0:],
                    start=(jb == 0), stop=(jb == njb - 1),
                )

        def finish(it, pair, ctx_t):
            recs = pb.tile([128, 2, 512], FP32, name="recs", tag="recs",
                           bufs=2)
            nc.vector.reciprocal_approx_fast(recs[:], ctx_t[:])
            loads["v"] += cost("v", 1024)
            rcn = pb.tile([128, 512], FP32, name="rec_n", tag="rec_n",
                          bufs=2)
            # half-crossing DMAs from the GpSimd queue (whose multiply
            # consumes them next) so they never head-of-line block the Sync
            # queue's output DMAs.
            nc.gpsimd.dma_start(rcn[0:64, :], recs[64:128, 0, :])
            nc.gpsimd.dma_start(rcn[64:128, :], recs[0:64, 1, :])
            # merged = ctx * rec; hl0 on ScalarE+GpSimd, hl1 on VectorE
            # (straight off PSUM) so the two halves run concurrently.
            m0 = merged[pair][0:64, it]
            nc.scalar.copy(m0, ctx_t[0:64, 0, :])
            nc.gpsimd.tensor_tensor(out=m0, in0=m0, in1=rcn[0:64, :],
                                    op=ALU.mult)
            loads["s"] += cost("s", 512)
            loads["g"] += cost("g", 512)
            nc.vector.tensor_tensor(out=merged[pair][64:128, it],
                                    in0=ctx_t[64:128, 1, :],
                                    in1=rcn[64:128, :], op=ALU.mult)
            loads["v"] += cost("v", 512)

        def emit_C_group(it_, gi):
            tpc = tileAB(f"c{it_}_{gi}")
            for pair in range(NPAIR):
                for e2 in range(2):
                    eb = 2 * gi + e2
                    nc.tensor.matmul(
                        tpc[:, e2],
                        lhsT=wout_sb[:, pair, 128 * eb: 128 * (eb + 1)],
                        rhs=merged[pair][:, it_],
                        start=(pair == 0), stop=(pair == NPAIR - 1),
                    )
            osb = pb.tile([128, 2, 512], BF16, name="osb", tag="osb", bufs=6)
            drain(osb[:], tpc[:], 1024)
            nc.sync.dma_start(
                out_r[:, 2 * gi: 2 * gi + 2, 512 * it_: 512 * (it_ + 1)],
                osb[:],
            )

        L = 14
        cq = []
        for it in range(IT):
            njb = 4 * it + 4
            for pair in range(NPAIR):
                ctx_tag = "CD"[nCD[0] % 2]
                ctx_t = tileCD(f"ctx{it}_{pair}")
                # the vacated CD tag is still draining through the previous
                # pair's finish chain; let it re-enter the S ring only a few
                # slots into this run.
                free_cd[0] = None
                for jb in range(njb):
                    n = av_n[0]
                    if jb < 4 * it:
                        full_freed[0] += 1
                    emit_AV(it, pair, jb, pTs.pop(n), ctx_t)
                    av_n[0] += 1
                    if jb == 2:
                        free_cd[0] = f"ps{'DC'['CD'.index(ctx_tag)]}"
                    top_up(n + 1 + L)
                    # hold back two C groups before the last it so the PE has
                    # ready work to chew on during the final finish chain
                    if cq and jb % 4 == 3 and (it < 3 or len(cq) > 2):
                        emit_C_group(*cq.pop(0))
                finish(it, pair, ctx_t)
                if pair == NPAIR - 1:
                    for gi in range(4):
                        cq.append((it, gi))
        while cq:
            emit_C_group(*cq.pop(0))

    if compile:
        nc.compile()
    return nc


_PROGRAM = None


def _get_program():
    global _PROGRAM
    if _PROGRAM is None:
        _PROGRAM = build_program()
    return _PROGRAM


def _tri():
    dj = np.arange(128)[:, None]
    di = np.arange(128)[None, :]
    t = (dj <= di).astype(ml_dtypes.bfloat16)
    return np.ascontiguousarray(np.concatenate([t, t], axis=1))


def make_in_maps(x, Wqkv, Wout):
    in_maps = []
    for core in range(NCORES):
        b, hg = core // (NCORES // B), core % (NCORES // B)
        c0 = hg * HPC * Dh
        csl = slice(c0, c0 + HPC * Dh)
        wqk_full = np.concatenate(
            [Wqkv[:, csl], Wqkv[:, D + c0: D + c0 + HPC * Dh]], axis=1
        ).astype(ml_dtypes.bfloat16)
        wv_full = Wqkv[:, 2 * D + c0: 2 * D + c0 + HPC * Dh].astype(
            ml_dtypes.bfloat16)
        in_maps.append({
            "tri": _tri(),
            "xT": np.ascontiguousarray(x[b].T).astype(ml_dtypes.bfloat16),
            "wqk": np.ascontiguousarray(
                wqk_full.reshape(KO, 128, 2 * HPC * Dh).transpose(1, 0, 2)),
            "wv": np.ascontiguousarray(
                wv_full.reshape(KO, 128, HPC * Dh).transpose(1, 0, 2)),
            "wout": np.ascontiguousarray(
                Wout[csl, :].astype(ml_dtypes.bfloat16)
                .reshape(2, 128, D).transpose(1, 0, 2)),
        })
    return in_maps


def kernel(x, causal_mask, key_padding_mask, Wqkv, bqkv, Wout, bout,
           _trace=False):
    from concourse.bass_utils import run_bass_kernel_spmd

    x = np.asarray(x, dtype=np.float32)
    Wqkv = np.asarray(Wqkv, dtype=np.float32)
    Wout = np.asarray(Wout, dtype=np.float32)
    bqkv = np.asarray(bqkv, dtype=np.float32)
    bout = np.asarray(bout, dtype=np.float32)
    if np.any(np.asarray(key_padding_mask)):
        raise NotImplementedError("key_padding_mask with padded keys")
    if np.any(bqkv):
        raise NotImplementedError("nonzero bqkv")

    nc = _get_program()
    in_maps = make_in_maps(x, Wqkv, Wout)
    res = run_bass_kernel_spmd(nc, in_maps, core_ids=list(range(NCORES)),
                               trace=_trace)
    G = NCORES // B
    outp = np.empty((B, T, D), dtype=np.float32)
    for b in range(B):
        acc = res.results[b * G]["outT"].astype(np.float32)
        for hg in range(1, G):
            acc += res.results[b * G + hg]["outT"].astype(np.float32)
        outp[b] = acc.T + bout
    kernel.last_exec_time_ns = res.exec_time_ns
    return outp
